# revision 52
# baseline (speedup 1.0000x reference)
"""Multi-head causal attention on 8 Trainium2 NeuronCores.

Sharding: core c -> batch b = c // 4, head-group g = c % 4 (4 of 16 heads).
Each core computes its 4 heads' attention and the partial W_O contraction;
the host sums the 4 head-group partials per batch (the reduce of the
tensor-parallel split).

Device-side layout is transpose-free: the host pre-transposes x and the
weights so every matmul contraction lands on the partition axis:
  qT[e,s], kT[e,s]  = W^T-chunk.T @ xT-chunk          (accum over d)
  v[m,he]           = xT-chunk.T @ WvT-chunk          (accum over d)
  sT[m,s]           = kT-slice.T @ qT-block           (scores, transposed)
  pT[m,s]           = exp(sT * 1/sqrt(e))  * mask     (ScalarE + DVE)
  zT[e,s]          += v-slice.T @ pT                  (accum over m)
  den[1,s]         += ones.T @ pT                     (softmax denominator)
  recip             = reciprocal(den)                 (DVE)
  zn[e,s]           = zT * (ones x recip)             (PE outer-prod bcast)
  out[s,d]         += zn-slice.T @ WoT                (accum over heads)

Schedule: a single PSUM pool set ({gen:2, s:4, z:2} = 8 banks) is live
for the whole kernel, so there are no pool-close barriers.  The j=0
projections run as column-interleaved waves (7/5 concurrent PSUM chains)
so the PE consumes DMA chunks as they arrive; input DMAs are issued in
first-use order at 1-4 chunk granularity across the two HWDGE queues
(sync + scalar), since DMA engines drain descriptors in arrival order
and one big DMA blocks its sequencer ~0.65us per 128 descriptors.  For
j >= 1, the j-block's projections are emitted interleaved with the
(j-1)-block's attention and output projection.

Engine queues execute in-order, so latency chains are emitted where
other engines' work hides them: each head's softmax denominator matmul
+ reciprocal (part 1) and recip-broadcast + normalize (part 2) are
deferred into the NEXT head's first/third score steps; the final
block's heads 0-2 output pass runs as 3-matmul tiles pulled into the
last head's chunk stream.  The last-head combine is split: rows 0-1 sum
head 3 on the DVE (plain DMAs), rows 2-3 via gpsimd accumulate-DMAs
into DRAM (read-modify-write runs at ~1/3 rate, so only half the block
takes it) -- two independent tail streams.  Warm-up blips on a memset
tile open the HAM clock gate before the first projection wave.

All matmul operands fp16 (full PE rate), accumulation fp32 in PSUM.
"""

import math

import numpy as np

B = 2
S = 2048
D = 2048
H = 16
E = 128
HPC = 4          # heads per core
HE = HPC * E     # 512
NC_CHUNKS = D // 128   # 16 contraction chunks of 128
NBLK = 4         # s-blocks of 512
NMT = S // 128   # 16 m-tiles of 128
SCALE = 1.0 / math.sqrt(E)
N_CORES = 8

_CACHE = {}


def _build_program():
    import concourse.bacc as bacc
    import concourse.mybir as mybir
    import concourse.tile as tile

    f16 = mybir.dt.float16
    f32 = mybir.dt.float32
    Exp = mybir.ActivationFunctionType.Exp

    nc = bacc.Bacc("TRN2", target_bir_lowering=False, debug=False,
                   num_devices=N_CORES)

    xT_d = nc.dram_tensor("xT", [D, S], f16, kind="ExternalInput")
    wq_d = nc.dram_tensor("wq", [D, HE], f16, kind="ExternalInput")
    wk_d = nc.dram_tensor("wk", [D, HE], f16, kind="ExternalInput")
    wv_d = nc.dram_tensor("wv", [D, HE], f16, kind="ExternalInput")
    woT_d = nc.dram_tensor("woT", [HE, D], f16, kind="ExternalInput")
    masks_d = nc.dram_tensor("masks", [128, 128], f16, kind="ExternalInput")
    outp_d = nc.dram_tensor("outp", [S, D], f16, kind="ExternalOutput")

    with tile.TileContext(nc) as tc:
        with (
            tc.tile_pool(name="const", bufs=1) as constp,
            tc.tile_pool(name="qkv", bufs=1) as qkvp,
            tc.tile_pool(name="small", bufs=1) as smallp,
            tc.tile_pool(name="pt", bufs=7) as ptp,
            tc.tile_pool(name="post", bufs=1) as postp,
            tc.tile_pool(name="work", bufs=2) as workp,
            tc.tile_pool(name="osb", bufs=4) as osbp,
            tc.tile_pool(name="ps", bufs=1, space="PSUM") as psp,
        ):
            def ps_gen():
                return psp.tile([128, 512], f32, tag="gen", name="gen", bufs=2)

            def ps_s():
                return psp.tile([128, 512], f32, tag="s", name="sps", bufs=4)

            def ps_z():
                return psp.tile([128, 512], f32, tag="z", name="zps", bufs=2)

            # ones via memset (no DMA): blips + denominator lhsT can run
            # before any input lands, so the PE is HAM-warm by wave-A.
            ones_sb = constp.tile([128, 129], f16, tag="ones")
            nc.gpsimd.memset(ones_sb[:], 1.0)
            bwide = constp.tile([128, 512], f16, tag="bwide")
            nc.gpsimd.memset(bwide[:], 1.0)
            onesm = ones_sb[:, 0:1]            # [128, 1] denominator lhsT
            onescol = ones_sb[0:1, 1:129]      # [1, 128] broadcast lhsT
            woT_sb = constp.tile([128, HPC, D], f16, tag="woT")
            masks_sb = smallp.tile([128, 128], f16, tag="masks")

            qT = [qkvp.tile([128, S], f16, tag=f"qT{h}", name=f"qT{h}")
                  for h in range(HPC)]
            kT = [qkvp.tile([128, S], f16, tag=f"kT{h}", name=f"kT{h}")
                  for h in range(HPC)]
            vt = [qkvp.tile([128, HE], f16, tag=f"v{m}", name=f"v{m}")
                  for m in range(NMT)]

            # Pre-load the ScalarE Exp table while the first DMAs run, so
            # the first attention exp doesn't stall on ACT_TABLE_LOAD.
            # Seeded from a memset tile so the table load doesn't wait on
            # any DMA.
            tseed = workp.tile([1, 1], f16, tag="tseed", bufs=1)
            nc.gpsimd.memset(tseed[:], 1.0)
            twarm = workp.tile([1, 1], f16, tag="twarm", bufs=1)
            nc.scalar.activation(twarm[:], tseed[:], Exp, scale=1.0)
            # PE warm-up blips on the ones tile bridge the DMA lead-in so
            # the HAM clock gate opens before the projection stream starts.
            for w in range(11):
                wps = ps_gen()
                nc.tensor.matmul(wps[0:1, 0:512], lhsT=onesm,
                                 rhs=bwide[:], start=True, stop=True)

            zn = [[None] * NBLK for _ in range(HPC)]
            # per-partition reciprocal for the final head's tail combine
            recT_sb = workp.tile([128, 4], f32, tag="recT", bufs=1)

            def attn_block_gen(j, two_phase_out=False, heads=None):
                """Attention for block j, baseline inner pipeline;
                yields a PE-time weight after each step."""
                ph1 = None
                ph1_step = 0
                pending_norm = [None, None]   # [part1, part2]

                def run_pending(part):
                    if pending_norm[part] is not None:
                        pending_norm[part]()
                        pending_norm[part] = None

                hlist = list(heads if heads is not None else range(HPC))
                for h in hlist:
                    zps = ps_z()
                    nchunks = 4 * j + 4
                    pts = [None] * nchunks
                    cols = [None] * nchunks

                    def emit_score(i):
                        # columns < c0 are fully masked (never read)
                        r = i - 4 * j
                        c0 = 128 * r if r > 0 else 0
                        cols[i] = c0
                        sps = ps_s()
                        nc.tensor.matmul(
                            sps[:, c0:512],
                            lhsT=kT[h][:, i * 128:(i + 1) * 128],
                            rhs=qT[h][:, j * 512 + c0:(j + 1) * 512],
                            start=True, stop=True)
                        pt = ptp.tile([128, 512], f16, tag="pt",
                                      name="pt")
                        nc.scalar.activation(pt[:, c0:512],
                                             sps[:, c0:512], Exp,
                                             scale=SCALE)
                        if r >= 0:
                            # only the 128-wide diagonal band is
                            # partially masked
                            nc.vector.tensor_mul(
                                pt[:, c0:c0 + 128], pt[:, c0:c0 + 128],
                                masks_sb[:])
                        if r in (1, 3):
                            # zero the dead band so this chunk can be
                            # pair-summed with its even partner for
                            # the denominator
                            nc.gpsimd.memset(pt[:, c0 - 128:c0], 0.0)
                        pts[i] = pt

                    # denominator tree: chunk pairs on GpSimd, pair-sums
                    # combined on DVE into quads/octs so each ones-matmul
                    # covers 4-8 chunks; the 4 diagonal chunks merge into
                    # one full-width matmul.
                    den_state = {"held": None, "heldq": None,
                                 "quads": [], "octs": [],
                                 "dsum01": None, "held_d": None}

                    def emit_pv(i):
                        c0 = cols[i]
                        pt = pts[i]
                        last = (i == nchunks - 1)
                        nc.tensor.matmul(
                            zps[:, c0:512],
                            lhsT=vt[i][:, h * E:(h + 1) * E],
                            rhs=pt[:, c0:512], start=(i == 0), stop=last,
                            skip_group_check=(c0 > 0))
                        if i < 4 * j:
                            if den_state["held"] is None:
                                den_state["held"] = pt
                            else:
                                ptsum = workp.tile([128, 512], f16,
                                                   tag="ptsum",
                                                   name="ptsum",
                                                   bufs=3)
                                nc.gpsimd.tensor_add(
                                    ptsum[:], den_state["held"][:],
                                    pt[:])
                                den_state["held"] = None
                                if den_state["heldq"] is None:
                                    den_state["heldq"] = ptsum
                                else:
                                    qsum = workp.tile([128, 512], f16,
                                                      tag="qsum",
                                                      name="qsum",
                                                      bufs=3)
                                    nc.vector.tensor_add(
                                        qsum[:],
                                        den_state["heldq"][:],
                                        ptsum[:])
                                    den_state["heldq"] = None
                                    den_state["quads"].append(qsum)
                                    if len(den_state["quads"]) == 2:
                                        qa, qb = den_state["quads"]
                                        oct_ = workp.tile(
                                            [128, 512], f16, tag="oct",
                                            name="oct", bufs=1)
                                        nc.vector.tensor_add(
                                            oct_[:], qa[:], qb[:])
                                        den_state["quads"] = []
                                        den_state["octs"].append(oct_)
                        elif (i - 4 * j) in (0, 2):
                            den_state["held_d"] = pt
                        elif i - 4 * j == 1:
                            d01 = workp.tile([128, 512], f16,
                                             tag="dsum", name="dsum01")
                            nc.vector.tensor_add(
                                d01[:], den_state["held_d"][:], pt[:])
                            den_state["held_d"] = None
                            den_state["dsum01"] = d01
                        else:  # r == 3: fold r2+r3 into dsum01, one MM
                            d01 = den_state["dsum01"]
                            d23 = workp.tile([128, 512], f16,
                                             tag="dsum23", name="dsum23",
                                             bufs=1)
                            nc.vector.tensor_add(
                                d23[:, 256:512],
                                den_state["held_d"][:, 256:512],
                                pt[:, 256:512])
                            den_state["held_d"] = None
                            nc.vector.tensor_add(
                                d01[:, 256:512], d01[:, 256:512],
                                d23[:, 256:512])
                            # fold leftover quads/octs so ONE den
                            # matmul covers the whole head (frees the
                            # dedicated dps PSUM bank -> deeper s pool)
                            for q in (den_state["quads"]
                                      + den_state["octs"]):
                                nc.vector.tensor_add(
                                    d01[:], d01[:], q[:])
                            den_state["quads"] = []
                            den_state["octs"] = []
                        pts[i] = None

                    def pull_ph1():
                        # spread the final block's out tiles into the
                        # later heads' chunk streams
                        nonlocal ph1, ph1_step
                        if ph1 is not None:
                            ph1_step += 1
                            if next(ph1, None) is None:
                                ph1 = None

                    off = min(3, nchunks - 1)
                    for i in range(nchunks):
                        emit_score(i)
                        if i == 0:
                            # the previous head's denominator matmul +
                            # reciprocal run here, hidden behind this
                            # head's first scores (engine queues are
                            # in-order: emitted earlier would stall the
                            # PE on the DVE chain)
                            run_pending(0)
                        elif i == 2:
                            # ... and the recip-broadcast two steps
                            # later, once the reciprocal has drained
                            run_pending(1)
                        if i >= off:
                            emit_pv(i - off)
                        if i >= 3 and (i < nchunks - 5
                                       or h != hlist[-1]):
                            # (zn of the previous head materializes at
                            # i == 2 via run_pending(1)); the last head
                            # reserves 8 tiles for the normalize weave
                            pull_ph1()
                        yield 9
                    for i in range(nchunks - off, nchunks):
                        yield 5
                        if h != hlist[-1]:
                            # reserve the last tiles for the normalize
                            # weave below
                            pull_ph1()
                        emit_pv(i)
                    def make_norm(h_, zps_, d01_, tail_):
                        rec = (None if tail_ else
                               workp.tile([1, 512], f16, tag="rec"))

                        def fin1():
                            if tail_:
                                # transposed denominator: 4 tiny matmuls
                                # d01^T @ ones give den per PARTITION, so
                                # the tail normalizes with per-partition
                                # scalars -- no broadcast matmul, no bsb,
                                # and a [128,4] reciprocal (130ns).
                                dpsT = ps_s()
                                for k in range(4):
                                    nc.tensor.matmul(
                                        dpsT[:, k:k + 1],
                                        lhsT=d01_[:, 128 * k:
                                                  128 * k + 128],
                                        rhs=onesm,
                                        start=(k == 0), stop=(k == 3),
                                        skip_group_check=(k > 0))
                                nc.vector.reciprocal_approx_fast(
                                    recT_sb[:], dpsT[:, 0:4])
                                return
                            dps = ps_s()
                            nc.tensor.matmul(dps[0:1, :], lhsT=onesm,
                                             rhs=d01_[:],
                                             start=True, stop=True)
                            rec32 = workp.tile([1, 512], f32,
                                               tag="rec32", bufs=1)
                            nc.vector.reciprocal_approx_fast(
                                rec32[:], dps[0:1, :])
                            nc.vector.tensor_copy(rec[:], rec32[:])

                        def fin2():
                            if tail_:
                                # unnormalized cast, sliced per st in the
                                # tail's consumption order so the first
                                # out matmul starts after ~100ns
                                z = postp.tile([128, 512], f16,
                                               tag=f"zn{h_}_{j}",
                                               name=f"zn{h_}_{j}")
                                for stc in (0, 2, 1, 3):
                                    nc.vector.tensor_copy(
                                        z[:, 128 * stc:128 * stc + 128],
                                        zps_[:, 128 * stc:
                                             128 * stc + 128])
                                zn[h_][j] = z
                                return
                            bps = ps_s()
                            nc.tensor.matmul(bps[:], lhsT=onescol,
                                             rhs=rec[:],
                                             start=True, stop=True)
                            bsb = workp.tile([128, 512], f16, tag="bsb")
                            nc.vector.tensor_copy(bsb[:], bps[:])
                            z = postp.tile([128, 512], f16,
                                           tag=f"zn{h_}_{j}",
                                           name=f"zn{h_}_{j}")
                            nc.vector.tensor_mul(z[:], zps_[:], bsb[:])
                            zn[h_][j] = z
                        return fin1, fin2

                    pending_norm[0], pending_norm[1] = make_norm(
                        h, zps, den_state["dsum01"],
                        two_phase_out and h == hlist[-1])
                    yield 9
                    if two_phase_out and h == 2:
                        ph1 = _out_phase12_gen(j)
                        ph1_step = 0
                # last head: weave leftover out-tiles between the
                # denominator matmul and the broadcast so the PE is not
                # idle while the reciprocal drains on the DVE
                run_pending(0)
                for _ in range(4):
                    if ph1 is not None and next(ph1, None) is None:
                        ph1 = None
                run_pending(1)
                while ph1 is not None:
                    if next(ph1, None) is None:
                        ph1 = None

            osb12 = osbp.tile([128, 4, 4, 512], f16, tag="osb12",
                              name="osb12", bufs=1)

            def _out_row_dma(j, st, accum, half=None):
                # overwrite + accumulate row-DMAs share the gpsimd SWDGE
                # queue: single issue path => per-row ordering guaranteed
                # (only SWDGE can do accumulating DMAs).  half=0/1 sends
                # a [128,1024] half-row so the slow read-modify-write
                # transfer overlaps the remaining casts.
                row = j * 512 + st * 128
                op = (mybir.AluOpType.add if accum
                      else mybir.AluOpType.bypass)
                if half is None:
                    nc.gpsimd.dma_start(
                        outp_d[row:row + 128, :], osb12[:, st, :, :],
                        accum_op=op)
                else:
                    c0 = half * 1024
                    nc.gpsimd.dma_start(
                        outp_d[row:row + 128, c0:c0 + 1024],
                        osb12[:, st, 2 * half:2 * half + 2, :],
                        accum_op=op)

            def _out_phase12_gen(j):
                # heads 0-2 of the final out-block: pure-PE filler pulled
                # into the last attention head's chunk stream
                for st in range(4):
                    for db in range(4):
                        ops = ps_gen()
                        for h in range(3):
                            nc.tensor.matmul(
                                ops[:],
                                lhsT=zn[h][j][:, st * 128:(st + 1) * 128],
                                rhs=woT_sb[:, h, db * 512:(db + 1) * 512],
                                start=(h == 0), stop=(h == 2))
                        nc.vector.tensor_copy(osb12[:, st, db, :], ops[:])
                        if db == 3 and st >= 2:
                            # rows 2-3 take the DMA-accumulate path; rows
                            # 0-1 combine in SBUF at the tail instead
                            _out_row_dma(j, st, accum=False)
                        yield 7

            def out_block_gen(j, two_phase=False):
                for st in ([0, 2, 1, 3] if two_phase else range(4)):
                    for db in range(4):
                        # the tail runs after all attention: the 4-deep
                        # s pool is idle, use it for a deeper pipeline
                        ops = ps_s() if two_phase else ps_gen()
                        h0 = 3 if two_phase else 0
                        for h in range(h0, HPC):
                            nc.tensor.matmul(
                                ops[:],
                                lhsT=zn[h][j][:, st * 128:(st + 1) * 128],
                                rhs=woT_sb[:, h, db * 512:(db + 1) * 512],
                                start=(h == h0), stop=(h == HPC - 1))
                        if two_phase:
                            # split tail: accumulate-DMA moves data at
                            # ~1/3 rate (read-modify-write), so only rows
                            # 2-3 take it (ScalarE casts, it is exp-free
                            # now); rows 0-1 combine on the DVE and go
                            # out as plain DMAs -- two independent
                            # streams in parallel.
                            if st >= 2:
                                nc.scalar.mul(osb12[:, st, db, :],
                                              ops[:],
                                              recT_sb[:, st:st + 1])
                                if db == 3:
                                    _out_row_dma(j, st, accum=True)
                            else:
                                osb = osbp.tile([128, 512], f16,
                                                tag="osb", name="osb",
                                                bufs=6)
                                nc.vector.scalar_tensor_tensor(
                                    osb[:], ops[:],
                                    recT_sb[:, st:st + 1],
                                    osb12[:, st, db, :],
                                    mybir.AluOpType.mult,
                                    mybir.AluOpType.add)
                                row = j * 512 + st * 128
                                nc.sync.dma_start(
                                    outp_d[row:row + 128,
                                           db * 512:(db + 1) * 512],
                                    osb[:])
                            yield 5
                        else:
                            osb = osbp.tile([128, 512], f16, tag="osb",
                                            name="osb", bufs=6)
                            if (st * 4 + db) % 2 == 0:
                                nc.scalar.copy(osb[:], ops[:])
                            else:
                                nc.vector.tensor_copy(osb[:], ops[:])
                            row = j * 512 + st * 128
                            nc.sync.dma_start(
                                outp_d[row:row + 128,
                                       db * 512:(db + 1) * 512], osb[:])
                            yield 18

            def chain_gens(*gens):
                for g in gens:
                    for w in g:
                        yield w

            def drive(main_gen, main_total, side_gen, side_total):
                """Interleave side yields into main, paced by the
                PE-time weights both generators yield."""
                mw = 0
                sw = 0
                done = side_total == 0
                for w in main_gen:
                    mw += w
                    while not done and sw * main_total < mw * side_total:
                        try:
                            sw += next(side_gen)
                        except StopIteration:
                            done = True
                while not done:
                    try:
                        next(side_gen)
                    except StopIteration:
                        done = True

            # ---- Projections: xT + W streamed through a scoped pool
            with tc.tile_pool(name="big", bufs=1) as bigp:
                w_sb = {}
                for name in ("wq", "wk", "wv"):
                    w_sb[name] = bigp.tile([128, NC_CHUNKS, HE], f16,
                                           tag=name, name=name + "_sb")
                wsrc = {name: dram.rearrange("(c p) n -> p c n", p=128)
                        for name, dram in
                        (("wq", wq_d), ("wk", wk_d), ("wv", wv_d))}
                xsrc = xT_d.rearrange("(c p) s -> p c s", p=128)

                def xT_tile():
                    return bigp.tile([128, NC_CHUNKS, 512], f16,
                                     tag="xT", name="xT", bufs=2)

                # DMA emission in wave-A consumption order, with the
                # issue cost spread across three engine queues (sync,
                # scalar, vector) so the ~650ns-per-issue sequencer cost
                # doesn't pace the stream.  The back half of each tensor
                # goes out as one batched DMA (descriptors spread over
                # all 16 DMA engines either way).
                # The per-chunk j0 stream goes out FIRST, split across the
                # sync and scalar HWDGE queues (~650ns per issue each, so
                # the two queues together outpace the PE's ~1.5us/chunk
                # consumption).  Bulk transfers are emitted strictly
                # AFTER it: DMA engines drain descriptors in arrival
                # order, so early bulk would starve the chunk stream
                # (measured: 5.4us PE gap + HAM re-throttle).
                xT0 = xT_tile()
                for c in range(8):
                    qa = nc.sync if c % 2 == 0 else nc.scalar
                    qb = nc.scalar if c % 2 == 0 else nc.sync
                    qa.dma_start(w_sb["wk"][:, c:c + 1, :],
                                 wsrc["wk"][:, c:c + 1, :])
                    qb.dma_start(w_sb["wq"][:, c:c + 1, :],
                                 wsrc["wq"][:, c:c + 1, :])
                    qa.dma_start(xT0[:, c:c + 1, :],
                                 xsrc[:, c:c + 1, 0:512])
                # Everything else strictly AFTER the chunk stream, still
                # on the two HWDGE queues, in first-use order, at 2-4
                # chunk granularity: DIRECT2D blocks its sequencer for
                # ~0.65us per 128 descriptors, so one huge DMA would
                # stall the queue; and the 16 DMA engines drain their
                # descriptor FIFOs in arrival order, so emission order IS
                # arrival order.  No gpsimd SWDGE for inputs: its desc
                # generation runs as soon as the idle gpsimd sequencer
                # reaches it, jumping ahead of the chunk stream.
                alt = [nc.sync, nc.scalar]

                def adma(i, dst, src):
                    alt[i % 2].dma_start(dst, src)

                for g in range(4):
                    lo, hi = 8 + 2 * g, 10 + 2 * g
                    adma(g, xT0[:, lo:hi, :], xsrc[:, lo:hi, 0:512])
                    adma(g + 1, w_sb["wk"][:, lo:hi, :],
                         wsrc["wk"][:, lo:hi, :])
                    adma(g, w_sb["wq"][:, lo:hi, :],
                         wsrc["wq"][:, lo:hi, :])
                for g in range(4):
                    lo, hi = 4 * g, 4 * g + 4
                    adma(g, w_sb["wv"][:, lo:hi, :],
                         wsrc["wv"][:, lo:hi, :])
                nc.sync.dma_start(masks_sb[:], masks_d[:])
                xTb = [xT0, None, None, None]
                wot_src = woT_d.rearrange("(c p) d -> p c d", p=128)
                for j in range(1, NBLK):
                    xTb[j] = xT_tile()
                    for q in range(4):
                        adma(j + q, xTb[j][:, 4 * q:4 * q + 4, :],
                             xsrc[:, 4 * q:4 * q + 4,
                                  j * 512:(j + 1) * 512])
                nc.sync.dma_start(woT_sb[:, 0:2, :], wot_src[:, 0:2, :])
                nc.scalar.dma_start(woT_sb[:, 2:4, :], wot_src[:, 2:4, :])

                # ---- j0: column-interleaved waves so the PE consumes
                # chunks at DMA arrival rate from the first piece.
                def wave(chains, blips=0):
                    # chains: list of (psum_tile, lhsT_of_c, dst_ap).
                    # blips: extra tiny matmuls after the first steps —
                    # they fill the DMA-wait gaps so the HAM activity
                    # window stays busy and the clock gate opens early.
                    for c in range(NC_CHUNKS - 1):
                        for ps, lhs_of_c, _ in chains:
                            nc.tensor.matmul(
                                ps[:], lhsT=lhs_of_c(c),
                                rhs=xT0[:, c, :],
                                start=(c == 0), stop=False)
                        if c < blips:
                            for _ in range(2):
                                wps = ps_s()
                                nc.tensor.matmul(
                                    wps[0:1, 0:128], lhsT=onesm,
                                    rhs=ones_sb[:, 1:129],
                                    start=True, stop=True)
                    # last chunk + cast per chain, so the drains overlap
                    # the next wave's first matmuls instead of stalling it
                    for k, (ps, lhs_of_c, dst) in enumerate(chains):
                        nc.tensor.matmul(
                            ps[:], lhsT=lhs_of_c(NC_CHUNKS - 1),
                            rhs=xT0[:, NC_CHUNKS - 1, :],
                            start=False, stop=True)
                        if k % 2 == 0:
                            nc.scalar.copy(dst, ps[:])
                        else:
                            nc.vector.tensor_copy(dst, ps[:])

                def wslice(name, h):
                    return lambda c: w_sb[name][:, c, h * E:(h + 1) * E]

                waveA = []
                for h in range(HPC):
                    waveA.append((ps_gen() if h < 2 else ps_s(),
                                  wslice("wk", h), kT[h][:, 0:512]))
                for h in range(3):
                    waveA.append((ps_s() if h == 0 else ps_z(),
                                  wslice("wq", h), qT[h][:, 0:512]))
                wave(waveA, blips=4)

                # wave-B: qT h3 + the four v tiles of block 0.
                # v chains: lhsT = xT chunk slice, rhs = wv chunk.
                vB = []
                for m in range(4):
                    vB.append((ps_gen() if m < 2 else ps_s(), m))
                psq3 = ps_z()
                for c in range(NC_CHUNKS - 1):
                    nc.tensor.matmul(
                        psq3[:], lhsT=wslice("wq", 3)(c),
                        rhs=xT0[:, c, :],
                        start=(c == 0), stop=False)
                    for ps, m in vB:
                        nc.tensor.matmul(
                            ps[:], lhsT=xT0[:, c, m * 128:(m + 1) * 128],
                            rhs=w_sb["wv"][:, c, :],
                            start=(c == 0), stop=False)
                cl = NC_CHUNKS - 1
                nc.tensor.matmul(psq3[:], lhsT=wslice("wq", 3)(cl),
                                 rhs=xT0[:, cl, :], start=False, stop=True)
                nc.scalar.copy(qT[3][:, 0:512], psq3[:])
                for ps, m in vB:
                    nc.tensor.matmul(
                        ps[:], lhsT=xT0[:, cl, m * 128:(m + 1) * 128],
                        rhs=w_sb["wv"][:, cl, :], start=False, stop=True)
                    if m % 2 == 0:
                        nc.scalar.copy(vt[m][:], ps[:])
                    else:
                        nc.vector.tensor_copy(vt[m][:], ps[:])

                # ---- j1..j3 projections, interleaved with the previous
                # block's attention + output projection.
                def proj_block_gen(j):
                    def proj(dst_ap, lhs_of_c, rhs_of_c):
                        ps = ps_gen()
                        for c in range(NC_CHUNKS):
                            nc.tensor.matmul(
                                ps[:], lhsT=lhs_of_c(c), rhs=rhs_of_c(c),
                                start=(c == 0), stop=(c == NC_CHUNKS - 1))
                        nc.vector.tensor_copy(dst_ap, ps[:])

                    for h in range(HPC):
                        for dst, w in ((kT[h], "wk"), (qT[h], "wq")):
                            proj(dst[:, j * 512:(j + 1) * 512],
                                 wslice(w, h),
                                 lambda c: xTb[j][:, c, :])
                            yield 34
                    for m in range(4 * j, 4 * j + 4):
                        proj(vt[m][:],
                             lambda c, m=m: xTb[j][:, c,
                                                   (m - 4 * j) * 128:
                                                   (m - 4 * j + 1) * 128],
                             lambda c: w_sb["wv"][:, c, :])
                        yield 34

                # stages: proj(j) interleaved with attn(j-1) — the
                # out-blocks (and attn(2)'s last head, for balance) are
                # saved for the final stretch, where they are the
                # PE-dense filler for attn(3)'s exp-paced stream.
                for j in range(1, NBLK):
                    nheads = 3 if j == 3 else 4
                    nch = 4 * (j - 1) + 4
                    side_total = nheads * (nch * 9 + 3 * 5 + 9)
                    drive(proj_block_gen(j), 34 * 12,
                          attn_block_gen(j - 1, heads=range(nheads)),
                          side_total)

            # ---- final stretch: [attn(2)h3 + attn(3)] ∥ out(0..2),
            # then out(3) ph2.  Heads stay sequential (single dps bank).
            main = chain_gens(attn_block_gen(2, heads=[3]),
                              attn_block_gen(NBLK - 1,
                                             two_phase_out=True))
            side = chain_gens(out_block_gen(0), out_block_gen(1),
                              out_block_gen(2))
            nch = 4 * (NBLK - 1) + 4
            main_total = 4 * (nch * 9 + 3 * 5 + 9) + (12 * 9 + 3 * 5 + 9)
            drive(main, main_total, side, 48 * 18)
            for _ in out_block_gen(NBLK - 1, two_phase=True):
                pass

    nc.compile()
    return nc


def _get_nc():
    if "nc" not in _CACHE:
        _CACHE["nc"] = _build_program()
    return _CACHE["nc"]


def _host_inputs(x, W_Q, W_K, W_V, W_O):
    """Per-core input dicts (all fp16, pre-transposed)."""
    cc = np.arange(128)[None, :]
    mm = np.arange(128)[:, None]
    masks = (cc >= mm).astype(np.float16)   # [128,128] diagonal band
    in_maps = []
    for c in range(N_CORES):
        b, g = divmod(c, 4)
        hs = slice(HPC * g, HPC * g + HPC)
        xT = np.ascontiguousarray(x[b].T).astype(np.float16)
        wq = np.ascontiguousarray(
            W_Q[hs].transpose(2, 0, 1).reshape(D, HE)).astype(np.float16)
        wk = np.ascontiguousarray(
            W_K[hs].transpose(2, 0, 1).reshape(D, HE)).astype(np.float16)
        wv = np.ascontiguousarray(
            W_V[hs].transpose(2, 0, 1).reshape(D, HE)).astype(np.float16)
        woT = np.ascontiguousarray(
            W_O[hs].transpose(0, 2, 1).reshape(HE, D)).astype(np.float16)
        in_maps.append({"xT": xT, "wq": wq, "wk": wk, "wv": wv,
                        "woT": woT, "masks": masks})
    return in_maps


def _run(in_maps, trace=False, **kw):
    from concourse.bass_utils import run_bass_kernel_spmd
    nc = _get_nc()
    return run_bass_kernel_spmd(nc, in_maps, list(range(N_CORES)),
                                trace=trace, **kw)


def kernel(x, W_Q, W_K, W_V, W_O):
    x, W_Q, W_K, W_V, W_O = (np.asarray(a, dtype=np.float32)
                             for a in (x, W_Q, W_K, W_V, W_O))
    res = _run(_host_inputs(x, W_Q, W_K, W_V, W_O))
    parts = [np.asarray(res.results[c]["outp"], dtype=np.float32)
             for c in range(N_CORES)]
    out = np.stack([parts[0] + parts[1] + parts[2] + parts[3],
                    parts[4] + parts[5] + parts[6] + parts[7]])
    return out



# revision 53
# speedup vs baseline: 1.0000x; 1.0000x over previous
"""Multi-head causal attention on 8 Trainium2 NeuronCores.

Sharding: core c -> batch b = c // 4, head-group g = c % 4 (4 of 16 heads).
Each core computes its 4 heads' attention and the partial W_O contraction;
the host sums the 4 head-group partials per batch (the reduce of the
tensor-parallel split).

Device-side layout is transpose-free: the host pre-transposes x and the
weights so every matmul contraction lands on the partition axis:
  qT[e,s], kT[e,s]  = W^T-chunk.T @ xT-chunk          (accum over d)
  v[m,he]           = xT-chunk.T @ WvT-chunk          (accum over d)
  sT[m,s]           = kT-slice.T @ qT-block           (scores, transposed)
  pT[m,s]           = exp(sT * 1/sqrt(e))  * mask     (ScalarE + DVE)
  zT[e,s]          += v-slice.T @ pT                  (accum over m)
  den[1,s]         += ones.T @ pT                     (softmax denominator)
  recip             = reciprocal(den)                 (DVE)
  zn[e,s]           = zT * (ones x recip)             (PE outer-prod bcast)
  out[s,d]         += zn-slice.T @ WoT                (accum over heads)

Schedule: a single PSUM pool set ({gen:2, s:4, z:2} = 8 banks) is live
for the whole kernel, so there are no pool-close barriers.  The j=0
projections run as column-interleaved waves (7/5 concurrent PSUM chains)
so the PE consumes DMA chunks as they arrive; input DMAs are issued in
first-use order at 1-4 chunk granularity across the two HWDGE queues
(sync + scalar), since DMA engines drain descriptors in arrival order
and one big DMA blocks its sequencer ~0.65us per 128 descriptors.  For
j >= 1, the j-block's projections are emitted interleaved with the
(j-1)-block's attention and output projection.

Engine queues execute in-order, so latency chains are emitted where
other engines' work hides them: each head's softmax denominator matmul
+ reciprocal (part 1) and recip-broadcast + normalize (part 2) are
deferred into the NEXT head's first/third score steps; the final
block's heads 0-2 output pass runs as 3-matmul tiles pulled into the
last head's chunk stream.  The last-head combine is split: rows 0-1 sum
head 3 on the DVE (plain DMAs), rows 2-3 via gpsimd accumulate-DMAs
into DRAM (read-modify-write runs at ~1/3 rate, so only half the block
takes it) -- two independent tail streams.  Warm-up blips on a memset
tile open the HAM clock gate before the first projection wave.

All matmul operands fp16 (full PE rate), accumulation fp32 in PSUM.
"""

import math

import numpy as np

B = 2
S = 2048
D = 2048
H = 16
E = 128
HPC = 4          # heads per core
HE = HPC * E     # 512
NC_CHUNKS = D // 128   # 16 contraction chunks of 128
NBLK = 4         # s-blocks of 512
NMT = S // 128   # 16 m-tiles of 128
SCALE = 1.0 / math.sqrt(E)
N_CORES = 8

_CACHE = {}


def _build_program():
    import concourse.bacc as bacc
    import concourse.mybir as mybir
    import concourse.tile as tile

    f16 = mybir.dt.float16
    f32 = mybir.dt.float32
    Exp = mybir.ActivationFunctionType.Exp

    nc = bacc.Bacc("TRN2", target_bir_lowering=False, debug=False,
                   num_devices=N_CORES)

    xT_d = nc.dram_tensor("xT", [D, S], f16, kind="ExternalInput")
    wq_d = nc.dram_tensor("wq", [D, HE], f16, kind="ExternalInput")
    wk_d = nc.dram_tensor("wk", [D, HE], f16, kind="ExternalInput")
    wv_d = nc.dram_tensor("wv", [D, HE], f16, kind="ExternalInput")
    woT_d = nc.dram_tensor("woT", [HE, D], f16, kind="ExternalInput")
    masks_d = nc.dram_tensor("masks", [128, 128], f16, kind="ExternalInput")
    outp_d = nc.dram_tensor("outp", [S, D], f16, kind="ExternalOutput")

    with tile.TileContext(nc) as tc:
        with (
            tc.tile_pool(name="const", bufs=1) as constp,
            tc.tile_pool(name="qkv", bufs=1) as qkvp,
            tc.tile_pool(name="small", bufs=1) as smallp,
            tc.tile_pool(name="pt", bufs=7) as ptp,
            tc.tile_pool(name="post", bufs=1) as postp,
            tc.tile_pool(name="work", bufs=2) as workp,
            tc.tile_pool(name="osb", bufs=4) as osbp,
            tc.tile_pool(name="ps", bufs=1, space="PSUM") as psp,
        ):
            def ps_gen():
                return psp.tile([128, 512], f32, tag="gen", name="gen", bufs=2)

            def ps_s():
                return psp.tile([128, 512], f32, tag="s", name="sps", bufs=4)

            def ps_z():
                return psp.tile([128, 512], f32, tag="z", name="zps", bufs=2)

            # ones via memset (no DMA): blips + denominator lhsT can run
            # before any input lands, so the PE is HAM-warm by wave-A.
            ones_sb = constp.tile([128, 129], f16, tag="ones")
            nc.gpsimd.memset(ones_sb[:], 1.0)
            bwide = constp.tile([128, 512], f16, tag="bwide")
            nc.gpsimd.memset(bwide[:], 1.0)
            onesm = ones_sb[:, 0:1]            # [128, 1] denominator lhsT
            onescol = ones_sb[0:1, 1:129]      # [1, 128] broadcast lhsT
            woT_sb = constp.tile([128, HPC, D], f16, tag="woT")
            masks_sb = smallp.tile([128, 128], f16, tag="masks")

            qT = [qkvp.tile([128, S], f16, tag=f"qT{h}", name=f"qT{h}")
                  for h in range(HPC)]
            kT = [qkvp.tile([128, S], f16, tag=f"kT{h}", name=f"kT{h}")
                  for h in range(HPC)]
            vt = [qkvp.tile([128, HE], f16, tag=f"v{m}", name=f"v{m}")
                  for m in range(NMT)]

            # Pre-load the ScalarE Exp table while the first DMAs run, so
            # the first attention exp doesn't stall on ACT_TABLE_LOAD.
            # Seeded from a memset tile so the table load doesn't wait on
            # any DMA.
            tseed = workp.tile([1, 1], f16, tag="tseed", bufs=1)
            nc.gpsimd.memset(tseed[:], 1.0)
            twarm = workp.tile([1, 1], f16, tag="twarm", bufs=1)
            nc.scalar.activation(twarm[:], tseed[:], Exp, scale=1.0)
            # PE warm-up blips on the ones tile bridge the DMA lead-in so
            # the HAM clock gate opens before the projection stream starts.
            for w in range(10):
                wps = ps_gen()
                nc.tensor.matmul(wps[0:1, 0:512], lhsT=onesm,
                                 rhs=bwide[:], start=True, stop=True)

            zn = [[None] * NBLK for _ in range(HPC)]
            # per-partition reciprocal for the final head's tail combine
            recT_sb = workp.tile([128, 4], f32, tag="recT", bufs=1)

            def attn_block_gen(j, two_phase_out=False, heads=None):
                """Attention for block j, baseline inner pipeline;
                yields a PE-time weight after each step."""
                ph1 = None
                ph1_step = 0
                pending_norm = [None, None]   # [part1, part2]

                def run_pending(part):
                    if pending_norm[part] is not None:
                        pending_norm[part]()
                        pending_norm[part] = None

                hlist = list(heads if heads is not None else range(HPC))
                for h in hlist:
                    zps = ps_z()
                    nchunks = 4 * j + 4
                    pts = [None] * nchunks
                    cols = [None] * nchunks

                    def emit_score(i):
                        # columns < c0 are fully masked (never read)
                        r = i - 4 * j
                        c0 = 128 * r if r > 0 else 0
                        cols[i] = c0
                        sps = ps_s()
                        nc.tensor.matmul(
                            sps[:, c0:512],
                            lhsT=kT[h][:, i * 128:(i + 1) * 128],
                            rhs=qT[h][:, j * 512 + c0:(j + 1) * 512],
                            start=True, stop=True)
                        pt = ptp.tile([128, 512], f16, tag="pt",
                                      name="pt")
                        nc.scalar.activation(pt[:, c0:512],
                                             sps[:, c0:512], Exp,
                                             scale=SCALE)
                        if r >= 0:
                            # only the 128-wide diagonal band is
                            # partially masked
                            nc.vector.tensor_mul(
                                pt[:, c0:c0 + 128], pt[:, c0:c0 + 128],
                                masks_sb[:])
                        if r in (1, 3):
                            # zero the dead band so this chunk can be
                            # pair-summed with its even partner for
                            # the denominator
                            nc.gpsimd.memset(pt[:, c0 - 128:c0], 0.0)
                        pts[i] = pt

                    # denominator tree: chunk pairs on GpSimd, pair-sums
                    # combined on DVE into quads/octs so each ones-matmul
                    # covers 4-8 chunks; the 4 diagonal chunks merge into
                    # one full-width matmul.
                    den_state = {"held": None, "heldq": None,
                                 "quads": [], "octs": [],
                                 "dsum01": None, "held_d": None}

                    def emit_pv(i):
                        c0 = cols[i]
                        pt = pts[i]
                        last = (i == nchunks - 1)
                        nc.tensor.matmul(
                            zps[:, c0:512],
                            lhsT=vt[i][:, h * E:(h + 1) * E],
                            rhs=pt[:, c0:512], start=(i == 0), stop=last,
                            skip_group_check=(c0 > 0))
                        if i < 4 * j:
                            if den_state["held"] is None:
                                den_state["held"] = pt
                            else:
                                ptsum = workp.tile([128, 512], f16,
                                                   tag="ptsum",
                                                   name="ptsum",
                                                   bufs=3)
                                nc.gpsimd.tensor_add(
                                    ptsum[:], den_state["held"][:],
                                    pt[:])
                                den_state["held"] = None
                                if den_state["heldq"] is None:
                                    den_state["heldq"] = ptsum
                                else:
                                    qsum = workp.tile([128, 512], f16,
                                                      tag="qsum",
                                                      name="qsum",
                                                      bufs=3)
                                    nc.vector.tensor_add(
                                        qsum[:],
                                        den_state["heldq"][:],
                                        ptsum[:])
                                    den_state["heldq"] = None
                                    den_state["quads"].append(qsum)
                                    if len(den_state["quads"]) == 2:
                                        qa, qb = den_state["quads"]
                                        oct_ = workp.tile(
                                            [128, 512], f16, tag="oct",
                                            name="oct", bufs=1)
                                        nc.vector.tensor_add(
                                            oct_[:], qa[:], qb[:])
                                        den_state["quads"] = []
                                        den_state["octs"].append(oct_)
                        elif (i - 4 * j) in (0, 2):
                            den_state["held_d"] = pt
                        elif i - 4 * j == 1:
                            d01 = workp.tile([128, 512], f16,
                                             tag="dsum", name="dsum01")
                            nc.vector.tensor_add(
                                d01[:], den_state["held_d"][:], pt[:])
                            den_state["held_d"] = None
                            den_state["dsum01"] = d01
                        else:  # r == 3: fold r2+r3 into dsum01, one MM
                            d01 = den_state["dsum01"]
                            d23 = workp.tile([128, 512], f16,
                                             tag="dsum23", name="dsum23",
                                             bufs=1)
                            nc.vector.tensor_add(
                                d23[:, 256:512],
                                den_state["held_d"][:, 256:512],
                                pt[:, 256:512])
                            den_state["held_d"] = None
                            nc.vector.tensor_add(
                                d01[:, 256:512], d01[:, 256:512],
                                d23[:, 256:512])
                            # fold leftover quads/octs so ONE den
                            # matmul covers the whole head (frees the
                            # dedicated dps PSUM bank -> deeper s pool)
                            for q in (den_state["quads"]
                                      + den_state["octs"]):
                                nc.vector.tensor_add(
                                    d01[:], d01[:], q[:])
                            den_state["quads"] = []
                            den_state["octs"] = []
                        pts[i] = None

                    def pull_ph1():
                        # spread the final block's out tiles into the
                        # later heads' chunk streams
                        nonlocal ph1, ph1_step
                        if ph1 is not None:
                            ph1_step += 1
                            if next(ph1, None) is None:
                                ph1 = None

                    off = min(3, nchunks - 1)
                    for i in range(nchunks):
                        emit_score(i)
                        if i == 0:
                            # the previous head's denominator matmul +
                            # reciprocal run here, hidden behind this
                            # head's first scores (engine queues are
                            # in-order: emitted earlier would stall the
                            # PE on the DVE chain)
                            run_pending(0)
                        elif i == 2:
                            # ... and the recip-broadcast two steps
                            # later, once the reciprocal has drained
                            run_pending(1)
                        if i >= off:
                            emit_pv(i - off)
                        if i >= 3 and (i < nchunks - 5
                                       or h != hlist[-1]):
                            # (zn of the previous head materializes at
                            # i == 2 via run_pending(1)); the last head
                            # reserves 8 tiles for the normalize weave
                            pull_ph1()
                        yield 9
                    for i in range(nchunks - off, nchunks):
                        yield 5
                        if h != hlist[-1]:
                            # reserve the last tiles for the normalize
                            # weave below
                            pull_ph1()
                        emit_pv(i)
                    def make_norm(h_, zps_, d01_, tail_):
                        rec = (None if tail_ else
                               workp.tile([1, 512], f16, tag="rec"))

                        def fin1():
                            if tail_:
                                # transposed denominator: 4 tiny matmuls
                                # d01^T @ ones give den per PARTITION, so
                                # the tail normalizes with per-partition
                                # scalars -- no broadcast matmul, no bsb,
                                # and a [128,4] reciprocal (130ns).
                                dpsT = ps_s()
                                for k in range(4):
                                    nc.tensor.matmul(
                                        dpsT[:, k:k + 1],
                                        lhsT=d01_[:, 128 * k:
                                                  128 * k + 128],
                                        rhs=onesm,
                                        start=(k == 0), stop=(k == 3),
                                        skip_group_check=(k > 0))
                                nc.vector.reciprocal_approx_fast(
                                    recT_sb[:], dpsT[:, 0:4])
                                return
                            dps = ps_s()
                            nc.tensor.matmul(dps[0:1, :], lhsT=onesm,
                                             rhs=d01_[:],
                                             start=True, stop=True)
                            rec32 = workp.tile([1, 512], f32,
                                               tag="rec32", bufs=1)
                            nc.vector.reciprocal_approx_fast(
                                rec32[:], dps[0:1, :])
                            nc.vector.tensor_copy(rec[:], rec32[:])

                        def fin2():
                            if tail_:
                                # unnormalized cast, sliced per st in the
                                # tail's consumption order so the first
                                # out matmul starts after ~100ns
                                z = postp.tile([128, 512], f16,
                                               tag=f"zn{h_}_{j}",
                                               name=f"zn{h_}_{j}")
                                for stc in (0, 2, 1, 3):
                                    nc.vector.tensor_copy(
                                        z[:, 128 * stc:128 * stc + 128],
                                        zps_[:, 128 * stc:
                                             128 * stc + 128])
                                zn[h_][j] = z
                                return
                            bps = ps_s()
                            nc.tensor.matmul(bps[:], lhsT=onescol,
                                             rhs=rec[:],
                                             start=True, stop=True)
                            bsb = workp.tile([128, 512], f16, tag="bsb")
                            nc.vector.tensor_copy(bsb[:], bps[:])
                            z = postp.tile([128, 512], f16,
                                           tag=f"zn{h_}_{j}",
                                           name=f"zn{h_}_{j}")
                            nc.vector.tensor_mul(z[:], zps_[:], bsb[:])
                            zn[h_][j] = z
                        return fin1, fin2

                    pending_norm[0], pending_norm[1] = make_norm(
                        h, zps, den_state["dsum01"],
                        two_phase_out and h == hlist[-1])
                    yield 9
                    if two_phase_out and h == 2:
                        ph1 = _out_phase12_gen(j)
                        ph1_step = 0
                # last head: weave leftover out-tiles between the
                # denominator matmul and the broadcast so the PE is not
                # idle while the reciprocal drains on the DVE
                run_pending(0)
                for _ in range(4):
                    if ph1 is not None and next(ph1, None) is None:
                        ph1 = None
                run_pending(1)
                while ph1 is not None:
                    if next(ph1, None) is None:
                        ph1 = None

            osb12 = osbp.tile([128, 4, 4, 512], f16, tag="osb12",
                              name="osb12", bufs=1)

            def _out_row_dma(j, st, accum, half=None):
                # overwrite + accumulate row-DMAs share the gpsimd SWDGE
                # queue: single issue path => per-row ordering guaranteed
                # (only SWDGE can do accumulating DMAs).  half=0/1 sends
                # a [128,1024] half-row so the slow read-modify-write
                # transfer overlaps the remaining casts.
                row = j * 512 + st * 128
                op = (mybir.AluOpType.add if accum
                      else mybir.AluOpType.bypass)
                if half is None:
                    nc.gpsimd.dma_start(
                        outp_d[row:row + 128, :], osb12[:, st, :, :],
                        accum_op=op)
                else:
                    c0 = half * 1024
                    nc.gpsimd.dma_start(
                        outp_d[row:row + 128, c0:c0 + 1024],
                        osb12[:, st, 2 * half:2 * half + 2, :],
                        accum_op=op)

            def _out_phase12_gen(j):
                # heads 0-2 of the final out-block: pure-PE filler pulled
                # into the last attention head's chunk stream
                for st in range(4):
                    for db in range(4):
                        ops = ps_gen()
                        for h in range(3):
                            nc.tensor.matmul(
                                ops[:],
                                lhsT=zn[h][j][:, st * 128:(st + 1) * 128],
                                rhs=woT_sb[:, h, db * 512:(db + 1) * 512],
                                start=(h == 0), stop=(h == 2))
                        nc.vector.tensor_copy(osb12[:, st, db, :], ops[:])
                        if db == 3 and st >= 2:
                            # rows 2-3 take the DMA-accumulate path; rows
                            # 0-1 combine in SBUF at the tail instead
                            _out_row_dma(j, st, accum=False)
                        yield 7

            def out_block_gen(j, two_phase=False):
                for st in ([0, 2, 1, 3] if two_phase else range(4)):
                    for db in range(4):
                        # the tail runs after all attention: the 4-deep
                        # s pool is idle, use it for a deeper pipeline
                        ops = ps_s() if two_phase else ps_gen()
                        h0 = 3 if two_phase else 0
                        for h in range(h0, HPC):
                            nc.tensor.matmul(
                                ops[:],
                                lhsT=zn[h][j][:, st * 128:(st + 1) * 128],
                                rhs=woT_sb[:, h, db * 512:(db + 1) * 512],
                                start=(h == h0), stop=(h == HPC - 1))
                        if two_phase:
                            # split tail: accumulate-DMA moves data at
                            # ~1/3 rate (read-modify-write), so only rows
                            # 2-3 take it (ScalarE casts, it is exp-free
                            # now); rows 0-1 combine on the DVE and go
                            # out as plain DMAs -- two independent
                            # streams in parallel.
                            if st >= 2:
                                nc.scalar.mul(osb12[:, st, db, :],
                                              ops[:],
                                              recT_sb[:, st:st + 1])
                                if db == 3:
                                    _out_row_dma(j, st, accum=True)
                            else:
                                osb = osbp.tile([128, 512], f16,
                                                tag="osb", name="osb",
                                                bufs=6)
                                nc.vector.scalar_tensor_tensor(
                                    osb[:], ops[:],
                                    recT_sb[:, st:st + 1],
                                    osb12[:, st, db, :],
                                    mybir.AluOpType.mult,
                                    mybir.AluOpType.add)
                                row = j * 512 + st * 128
                                nc.sync.dma_start(
                                    outp_d[row:row + 128,
                                           db * 512:(db + 1) * 512],
                                    osb[:])
                            yield 5
                        else:
                            osb = osbp.tile([128, 512], f16, tag="osb",
                                            name="osb", bufs=6)
                            if (st * 4 + db) % 2 == 0:
                                nc.scalar.copy(osb[:], ops[:])
                            else:
                                nc.vector.tensor_copy(osb[:], ops[:])
                            row = j * 512 + st * 128
                            nc.sync.dma_start(
                                outp_d[row:row + 128,
                                       db * 512:(db + 1) * 512], osb[:])
                            yield 18

            def chain_gens(*gens):
                for g in gens:
                    for w in g:
                        yield w

            def drive(main_gen, main_total, side_gen, side_total):
                """Interleave side yields into main, paced by the
                PE-time weights both generators yield."""
                mw = 0
                sw = 0
                done = side_total == 0
                for w in main_gen:
                    mw += w
                    while not done and sw * main_total < mw * side_total:
                        try:
                            sw += next(side_gen)
                        except StopIteration:
                            done = True
                while not done:
                    try:
                        next(side_gen)
                    except StopIteration:
                        done = True

            # ---- Projections: xT + W streamed through a scoped pool
            with tc.tile_pool(name="big", bufs=1) as bigp:
                w_sb = {}
                for name in ("wq", "wk", "wv"):
                    w_sb[name] = bigp.tile([128, NC_CHUNKS, HE], f16,
                                           tag=name, name=name + "_sb")
                wsrc = {name: dram.rearrange("(c p) n -> p c n", p=128)
                        for name, dram in
                        (("wq", wq_d), ("wk", wk_d), ("wv", wv_d))}
                xsrc = xT_d.rearrange("(c p) s -> p c s", p=128)

                def xT_tile():
                    return bigp.tile([128, NC_CHUNKS, 512], f16,
                                     tag="xT", name="xT", bufs=2)

                # DMA emission in wave-A consumption order, with the
                # issue cost spread across three engine queues (sync,
                # scalar, vector) so the ~650ns-per-issue sequencer cost
                # doesn't pace the stream.  The back half of each tensor
                # goes out as one batched DMA (descriptors spread over
                # all 16 DMA engines either way).
                # The per-chunk j0 stream goes out FIRST, split across the
                # sync and scalar HWDGE queues (~650ns per issue each, so
                # the two queues together outpace the PE's ~1.5us/chunk
                # consumption).  Bulk transfers are emitted strictly
                # AFTER it: DMA engines drain descriptors in arrival
                # order, so early bulk would starve the chunk stream
                # (measured: 5.4us PE gap + HAM re-throttle).
                xT0 = xT_tile()
                for c in range(8):
                    qa = nc.sync if c % 2 == 0 else nc.scalar
                    qb = nc.scalar if c % 2 == 0 else nc.sync
                    qa.dma_start(w_sb["wk"][:, c:c + 1, :],
                                 wsrc["wk"][:, c:c + 1, :])
                    qb.dma_start(w_sb["wq"][:, c:c + 1, :],
                                 wsrc["wq"][:, c:c + 1, :])
                    qa.dma_start(xT0[:, c:c + 1, :],
                                 xsrc[:, c:c + 1, 0:512])
                # Everything else strictly AFTER the chunk stream, still
                # on the two HWDGE queues, in first-use order, at 2-4
                # chunk granularity: DIRECT2D blocks its sequencer for
                # ~0.65us per 128 descriptors, so one huge DMA would
                # stall the queue; and the 16 DMA engines drain their
                # descriptor FIFOs in arrival order, so emission order IS
                # arrival order.  No gpsimd SWDGE for inputs: its desc
                # generation runs as soon as the idle gpsimd sequencer
                # reaches it, jumping ahead of the chunk stream.
                alt = [nc.sync, nc.scalar]

                def adma(i, dst, src):
                    alt[i % 2].dma_start(dst, src)

                for g in range(4):
                    lo, hi = 8 + 2 * g, 10 + 2 * g
                    adma(g, xT0[:, lo:hi, :], xsrc[:, lo:hi, 0:512])
                    adma(g + 1, w_sb["wk"][:, lo:hi, :],
                         wsrc["wk"][:, lo:hi, :])
                    adma(g, w_sb["wq"][:, lo:hi, :],
                         wsrc["wq"][:, lo:hi, :])
                for g in range(4):
                    lo, hi = 4 * g, 4 * g + 4
                    adma(g, w_sb["wv"][:, lo:hi, :],
                         wsrc["wv"][:, lo:hi, :])
                nc.sync.dma_start(masks_sb[:], masks_d[:])
                xTb = [xT0, None, None, None]
                wot_src = woT_d.rearrange("(c p) d -> p c d", p=128)
                for j in range(1, NBLK):
                    xTb[j] = xT_tile()
                    for q in range(4):
                        adma(j + q, xTb[j][:, 4 * q:4 * q + 4, :],
                             xsrc[:, 4 * q:4 * q + 4,
                                  j * 512:(j + 1) * 512])
                nc.sync.dma_start(woT_sb[:, 0:2, :], wot_src[:, 0:2, :])
                nc.scalar.dma_start(woT_sb[:, 2:4, :], wot_src[:, 2:4, :])

                # ---- j0: column-interleaved waves so the PE consumes
                # chunks at DMA arrival rate from the first piece.
                def wave(chains, blips=0):
                    # chains: list of (psum_tile, lhsT_of_c, dst_ap).
                    # blips: extra tiny matmuls after the first steps —
                    # they fill the DMA-wait gaps so the HAM activity
                    # window stays busy and the clock gate opens early.
                    for c in range(NC_CHUNKS - 1):
                        for ps, lhs_of_c, _ in chains:
                            nc.tensor.matmul(
                                ps[:], lhsT=lhs_of_c(c),
                                rhs=xT0[:, c, :],
                                start=(c == 0), stop=False)
                        if c < blips:
                            for _ in range(2):
                                wps = ps_s()
                                nc.tensor.matmul(
                                    wps[0:1, 0:128], lhsT=onesm,
                                    rhs=ones_sb[:, 1:129],
                                    start=True, stop=True)
                    # last chunk + cast per chain, so the drains overlap
                    # the next wave's first matmuls instead of stalling it
                    for k, (ps, lhs_of_c, dst) in enumerate(chains):
                        nc.tensor.matmul(
                            ps[:], lhsT=lhs_of_c(NC_CHUNKS - 1),
                            rhs=xT0[:, NC_CHUNKS - 1, :],
                            start=False, stop=True)
                        if k % 2 == 0:
                            nc.scalar.copy(dst, ps[:])
                        else:
                            nc.vector.tensor_copy(dst, ps[:])

                def wslice(name, h):
                    return lambda c: w_sb[name][:, c, h * E:(h + 1) * E]

                waveA = []
                for h in range(HPC):
                    waveA.append((ps_gen() if h < 2 else ps_s(),
                                  wslice("wk", h), kT[h][:, 0:512]))
                for h in range(3):
                    waveA.append((ps_s() if h == 0 else ps_z(),
                                  wslice("wq", h), qT[h][:, 0:512]))
                wave(waveA, blips=4)

                # wave-B: qT h3 + the four v tiles of block 0.
                # v chains: lhsT = xT chunk slice, rhs = wv chunk.
                vB = []
                for m in range(4):
                    vB.append((ps_gen() if m < 2 else ps_s(), m))
                psq3 = ps_z()
                for c in range(NC_CHUNKS - 1):
                    nc.tensor.matmul(
                        psq3[:], lhsT=wslice("wq", 3)(c),
                        rhs=xT0[:, c, :],
                        start=(c == 0), stop=False)
                    for ps, m in vB:
                        nc.tensor.matmul(
                            ps[:], lhsT=xT0[:, c, m * 128:(m + 1) * 128],
                            rhs=w_sb["wv"][:, c, :],
                            start=(c == 0), stop=False)
                cl = NC_CHUNKS - 1
                nc.tensor.matmul(psq3[:], lhsT=wslice("wq", 3)(cl),
                                 rhs=xT0[:, cl, :], start=False, stop=True)
                nc.scalar.copy(qT[3][:, 0:512], psq3[:])
                for ps, m in vB:
                    nc.tensor.matmul(
                        ps[:], lhsT=xT0[:, cl, m * 128:(m + 1) * 128],
                        rhs=w_sb["wv"][:, cl, :], start=False, stop=True)
                    if m % 2 == 0:
                        nc.scalar.copy(vt[m][:], ps[:])
                    else:
                        nc.vector.tensor_copy(vt[m][:], ps[:])

                # ---- j1..j3 projections, interleaved with the previous
                # block's attention + output projection.
                def proj_block_gen(j):
                    def proj(dst_ap, lhs_of_c, rhs_of_c):
                        ps = ps_gen()
                        for c in range(NC_CHUNKS):
                            nc.tensor.matmul(
                                ps[:], lhsT=lhs_of_c(c), rhs=rhs_of_c(c),
                                start=(c == 0), stop=(c == NC_CHUNKS - 1))
                        nc.vector.tensor_copy(dst_ap, ps[:])

                    for h in range(HPC):
                        for dst, w in ((kT[h], "wk"), (qT[h], "wq")):
                            proj(dst[:, j * 512:(j + 1) * 512],
                                 wslice(w, h),
                                 lambda c: xTb[j][:, c, :])
                            yield 34
                    for m in range(4 * j, 4 * j + 4):
                        proj(vt[m][:],
                             lambda c, m=m: xTb[j][:, c,
                                                   (m - 4 * j) * 128:
                                                   (m - 4 * j + 1) * 128],
                             lambda c: w_sb["wv"][:, c, :])
                        yield 34

                # stages: proj(j) interleaved with attn(j-1) — the
                # out-blocks (and attn(2)'s last head, for balance) are
                # saved for the final stretch, where they are the
                # PE-dense filler for attn(3)'s exp-paced stream.
                for j in range(1, NBLK):
                    nheads = 3 if j == 3 else 4
                    nch = 4 * (j - 1) + 4
                    side_total = nheads * (nch * 9 + 3 * 5 + 9)
                    drive(proj_block_gen(j), 34 * 12,
                          attn_block_gen(j - 1, heads=range(nheads)),
                          side_total)

            # ---- final stretch: [attn(2)h3 + attn(3)] ∥ out(0..2),
            # then out(3) ph2.  Heads stay sequential (single dps bank).
            main = chain_gens(attn_block_gen(2, heads=[3]),
                              attn_block_gen(NBLK - 1,
                                             two_phase_out=True))
            side = chain_gens(out_block_gen(0), out_block_gen(1),
                              out_block_gen(2))
            nch = 4 * (NBLK - 1) + 4
            main_total = 4 * (nch * 9 + 3 * 5 + 9) + (12 * 9 + 3 * 5 + 9)
            drive(main, main_total, side, 48 * 18)
            for _ in out_block_gen(NBLK - 1, two_phase=True):
                pass

    nc.compile()
    return nc


def _get_nc():
    if "nc" not in _CACHE:
        _CACHE["nc"] = _build_program()
    return _CACHE["nc"]


def _host_inputs(x, W_Q, W_K, W_V, W_O):
    """Per-core input dicts (all fp16, pre-transposed)."""
    cc = np.arange(128)[None, :]
    mm = np.arange(128)[:, None]
    masks = (cc >= mm).astype(np.float16)   # [128,128] diagonal band
    in_maps = []
    for c in range(N_CORES):
        b, g = divmod(c, 4)
        hs = slice(HPC * g, HPC * g + HPC)
        xT = np.ascontiguousarray(x[b].T).astype(np.float16)
        wq = np.ascontiguousarray(
            W_Q[hs].transpose(2, 0, 1).reshape(D, HE)).astype(np.float16)
        wk = np.ascontiguousarray(
            W_K[hs].transpose(2, 0, 1).reshape(D, HE)).astype(np.float16)
        wv = np.ascontiguousarray(
            W_V[hs].transpose(2, 0, 1).reshape(D, HE)).astype(np.float16)
        woT = np.ascontiguousarray(
            W_O[hs].transpose(0, 2, 1).reshape(HE, D)).astype(np.float16)
        in_maps.append({"xT": xT, "wq": wq, "wk": wk, "wv": wv,
                        "woT": woT, "masks": masks})
    return in_maps


def _run(in_maps, trace=False, **kw):
    from concourse.bass_utils import run_bass_kernel_spmd
    nc = _get_nc()
    return run_bass_kernel_spmd(nc, in_maps, list(range(N_CORES)),
                                trace=trace, **kw)


def kernel(x, W_Q, W_K, W_V, W_O):
    x, W_Q, W_K, W_V, W_O = (np.asarray(a, dtype=np.float32)
                             for a in (x, W_Q, W_K, W_V, W_O))
    res = _run(_host_inputs(x, W_Q, W_K, W_V, W_O))
    parts = [np.asarray(res.results[c]["outp"], dtype=np.float32)
             for c in range(N_CORES)]
    out = np.stack([parts[0] + parts[1] + parts[2] + parts[3],
                    parts[4] + parts[5] + parts[6] + parts[7]])
    return out



# revision 54
# speedup vs baseline: 1.0006x; 1.0006x over previous
"""Multi-head causal attention on 8 Trainium2 NeuronCores.

Sharding: core c -> batch b = c // 4, head-group g = c % 4 (4 of 16 heads).
Each core computes its 4 heads' attention and the partial W_O contraction;
the host sums the 4 head-group partials per batch (the reduce of the
tensor-parallel split).

Device-side layout is transpose-free: the host pre-transposes x and the
weights so every matmul contraction lands on the partition axis:
  qT[e,s], kT[e,s]  = W^T-chunk.T @ xT-chunk          (accum over d)
  v[m,he]           = xT-chunk.T @ WvT-chunk          (accum over d)
  sT[m,s]           = kT-slice.T @ qT-block           (scores, transposed)
  pT[m,s]           = exp(sT * 1/sqrt(e))  * mask     (ScalarE + DVE)
  zT[e,s]          += v-slice.T @ pT                  (accum over m)
  den[1,s]         += ones.T @ pT                     (softmax denominator)
  recip             = reciprocal(den)                 (DVE)
  zn[e,s]           = zT * (ones x recip)             (PE outer-prod bcast)
  out[s,d]         += zn-slice.T @ WoT                (accum over heads)

Schedule: a single PSUM pool set ({gen:2, s:4, z:2} = 8 banks) is live
for the whole kernel, so there are no pool-close barriers.  The j=0
projections run as column-interleaved waves (7/5 concurrent PSUM chains)
so the PE consumes DMA chunks as they arrive; input DMAs are issued in
first-use order at 1-4 chunk granularity across the two HWDGE queues
(sync + scalar), since DMA engines drain descriptors in arrival order
and one big DMA blocks its sequencer ~0.65us per 128 descriptors.  For
j >= 1, the j-block's projections are emitted interleaved with the
(j-1)-block's attention and output projection.

Engine queues execute in-order, so latency chains are emitted where
other engines' work hides them: each head's softmax denominator matmul
+ reciprocal (part 1) and recip-broadcast + normalize (part 2) are
deferred into the NEXT head's first/third score steps; the final
block's heads 0-2 output pass runs as 3-matmul tiles pulled into the
last head's chunk stream.  The last-head combine is split: rows 0-1 sum
head 3 on the DVE (plain DMAs), rows 2-3 via gpsimd accumulate-DMAs
into DRAM (read-modify-write runs at ~1/3 rate, so only half the block
takes it) -- two independent tail streams.  Warm-up blips on a memset
tile open the HAM clock gate before the first projection wave.

All matmul operands fp16 (full PE rate), accumulation fp32 in PSUM.
"""

import math

import numpy as np

B = 2
S = 2048
D = 2048
H = 16
E = 128
HPC = 4          # heads per core
HE = HPC * E     # 512
NC_CHUNKS = D // 128   # 16 contraction chunks of 128
NBLK = 4         # s-blocks of 512
NMT = S // 128   # 16 m-tiles of 128
SCALE = 1.0 / math.sqrt(E)
N_CORES = 8

_CACHE = {}


def _build_program():
    import concourse.bacc as bacc
    import concourse.mybir as mybir
    import concourse.tile as tile

    f16 = mybir.dt.float16
    f32 = mybir.dt.float32
    Exp = mybir.ActivationFunctionType.Exp

    nc = bacc.Bacc("TRN2", target_bir_lowering=False, debug=False,
                   num_devices=N_CORES)

    xT_d = nc.dram_tensor("xT", [D, S], f16, kind="ExternalInput")
    wq_d = nc.dram_tensor("wq", [D, HE], f16, kind="ExternalInput")
    wk_d = nc.dram_tensor("wk", [D, HE], f16, kind="ExternalInput")
    wv_d = nc.dram_tensor("wv", [D, HE], f16, kind="ExternalInput")
    woT_d = nc.dram_tensor("woT", [HE, D], f16, kind="ExternalInput")
    masks_d = nc.dram_tensor("masks", [128, 128], f16, kind="ExternalInput")
    outp_d = nc.dram_tensor("outp", [S, D], f16, kind="ExternalOutput")

    with tile.TileContext(nc) as tc:
        with (
            tc.tile_pool(name="const", bufs=1) as constp,
            tc.tile_pool(name="qkv", bufs=1) as qkvp,
            tc.tile_pool(name="small", bufs=1) as smallp,
            tc.tile_pool(name="pt", bufs=7) as ptp,
            tc.tile_pool(name="post", bufs=1) as postp,
            tc.tile_pool(name="work", bufs=2) as workp,
            tc.tile_pool(name="osb", bufs=4) as osbp,
            tc.tile_pool(name="ps", bufs=1, space="PSUM") as psp,
        ):
            def ps_gen():
                return psp.tile([128, 512], f32, tag="gen", name="gen", bufs=2)

            def ps_s():
                return psp.tile([128, 512], f32, tag="s", name="sps", bufs=4)

            def ps_z():
                return psp.tile([128, 512], f32, tag="z", name="zps", bufs=2)

            # ones via memset (no DMA): blips + denominator lhsT can run
            # before any input lands, so the PE is HAM-warm by wave-A.
            ones_sb = constp.tile([128, 129], f16, tag="ones")
            nc.gpsimd.memset(ones_sb[:], 1.0)
            bwide = constp.tile([128, 512], f16, tag="bwide")
            nc.gpsimd.memset(bwide[:], 1.0)
            onesm = ones_sb[:, 0:1]            # [128, 1] denominator lhsT
            onescol = ones_sb[0:1, 1:129]      # [1, 128] broadcast lhsT
            woT_sb = constp.tile([128, HPC, D], f16, tag="woT")
            masks_sb = smallp.tile([128, 128], f16, tag="masks")

            qT = [qkvp.tile([128, S], f16, tag=f"qT{h}", name=f"qT{h}")
                  for h in range(HPC)]
            kT = [qkvp.tile([128, S], f16, tag=f"kT{h}", name=f"kT{h}")
                  for h in range(HPC)]
            vt = [qkvp.tile([128, HE], f16, tag=f"v{m}", name=f"v{m}")
                  for m in range(NMT)]

            # Pre-load the ScalarE Exp table while the first DMAs run, so
            # the first attention exp doesn't stall on ACT_TABLE_LOAD.
            # Seeded from a memset tile so the table load doesn't wait on
            # any DMA.
            tseed = workp.tile([1, 1], f16, tag="tseed", bufs=1)
            nc.gpsimd.memset(tseed[:], 1.0)
            twarm = workp.tile([1, 1], f16, tag="twarm", bufs=1)
            nc.scalar.activation(twarm[:], tseed[:], Exp, scale=1.0)
            # PE warm-up blips on the ones tile bridge the DMA lead-in so
            # the HAM clock gate opens before the projection stream starts.
            for w in range(10):
                wps = ps_gen()
                nc.tensor.matmul(wps[0:1, 0:512], lhsT=onesm,
                                 rhs=bwide[:], start=True, stop=True)

            zn = [[None] * NBLK for _ in range(HPC)]
            # per-partition reciprocal for the final head's tail combine
            recT_sb = workp.tile([128, 4], f32, tag="recT", bufs=1)

            def attn_block_gen(j, two_phase_out=False, heads=None):
                """Attention for block j, baseline inner pipeline;
                yields a PE-time weight after each step."""
                ph1 = None
                ph1_step = 0
                pending_norm = [None, None]   # [part1, part2]

                def run_pending(part):
                    if pending_norm[part] is not None:
                        pending_norm[part]()
                        pending_norm[part] = None

                hlist = list(heads if heads is not None else range(HPC))
                for h in hlist:
                    zps = ps_z()
                    nchunks = 4 * j + 4
                    pts = [None] * nchunks
                    cols = [None] * nchunks

                    def emit_score(i):
                        # columns < c0 are fully masked (never read)
                        r = i - 4 * j
                        c0 = 128 * r if r > 0 else 0
                        cols[i] = c0
                        sps = ps_s()
                        nc.tensor.matmul(
                            sps[:, c0:512],
                            lhsT=kT[h][:, i * 128:(i + 1) * 128],
                            rhs=qT[h][:, j * 512 + c0:(j + 1) * 512],
                            start=True, stop=True)
                        pt = ptp.tile([128, 512], f16, tag="pt",
                                      name="pt")
                        nc.scalar.activation(pt[:, c0:512],
                                             sps[:, c0:512], Exp,
                                             scale=SCALE)
                        if r >= 0:
                            # only the 128-wide diagonal band is
                            # partially masked
                            nc.vector.tensor_mul(
                                pt[:, c0:c0 + 128], pt[:, c0:c0 + 128],
                                masks_sb[:])
                        if r in (1, 3):
                            # zero the dead band so this chunk can be
                            # pair-summed with its even partner for
                            # the denominator
                            nc.gpsimd.memset(pt[:, c0 - 128:c0], 0.0)
                        pts[i] = pt

                    # denominator tree: chunk pairs on GpSimd, pair-sums
                    # combined on DVE into quads/octs so each ones-matmul
                    # covers 4-8 chunks; the 4 diagonal chunks merge into
                    # one full-width matmul.
                    den_state = {"held": None, "heldq": None,
                                 "quads": [], "octs": [],
                                 "dsum01": None, "held_d": None}

                    def emit_pv(i):
                        c0 = cols[i]
                        pt = pts[i]
                        last = (i == nchunks - 1)
                        nc.tensor.matmul(
                            zps[:, c0:512],
                            lhsT=vt[i][:, h * E:(h + 1) * E],
                            rhs=pt[:, c0:512], start=(i == 0), stop=last,
                            skip_group_check=(c0 > 0))
                        if i < 4 * j:
                            if den_state["held"] is None:
                                den_state["held"] = pt
                            else:
                                ptsum = workp.tile([128, 512], f16,
                                                   tag="ptsum",
                                                   name="ptsum",
                                                   bufs=3)
                                nc.gpsimd.tensor_add(
                                    ptsum[:], den_state["held"][:],
                                    pt[:])
                                den_state["held"] = None
                                if den_state["heldq"] is None:
                                    den_state["heldq"] = ptsum
                                else:
                                    qsum = workp.tile([128, 512], f16,
                                                      tag="qsum",
                                                      name="qsum",
                                                      bufs=3)
                                    nc.vector.tensor_add(
                                        qsum[:],
                                        den_state["heldq"][:],
                                        ptsum[:])
                                    den_state["heldq"] = None
                                    den_state["quads"].append(qsum)
                                    if len(den_state["quads"]) == 2:
                                        qa, qb = den_state["quads"]
                                        oct_ = workp.tile(
                                            [128, 512], f16, tag="oct",
                                            name="oct", bufs=1)
                                        nc.vector.tensor_add(
                                            oct_[:], qa[:], qb[:])
                                        den_state["quads"] = []
                                        den_state["octs"].append(oct_)
                        elif (i - 4 * j) in (0, 2):
                            den_state["held_d"] = pt
                        elif i - 4 * j == 1:
                            d01 = workp.tile([128, 512], f16,
                                             tag="dsum", name="dsum01")
                            nc.vector.tensor_add(
                                d01[:], den_state["held_d"][:], pt[:])
                            den_state["held_d"] = None
                            den_state["dsum01"] = d01
                        else:  # r == 3: fold r2+r3 into dsum01, one MM
                            d01 = den_state["dsum01"]
                            d23 = workp.tile([128, 512], f16,
                                             tag="dsum23", name="dsum23",
                                             bufs=1)
                            nc.vector.tensor_add(
                                d23[:, 256:512],
                                den_state["held_d"][:, 256:512],
                                pt[:, 256:512])
                            den_state["held_d"] = None
                            nc.vector.tensor_add(
                                d01[:, 256:512], d01[:, 256:512],
                                d23[:, 256:512])
                            # fold leftover quads/octs so ONE den
                            # matmul covers the whole head (frees the
                            # dedicated dps PSUM bank -> deeper s pool)
                            for q in (den_state["quads"]
                                      + den_state["octs"]):
                                nc.vector.tensor_add(
                                    d01[:], d01[:], q[:])
                            den_state["quads"] = []
                            den_state["octs"] = []
                        pts[i] = None

                    def pull_ph1():
                        # spread the final block's out tiles into the
                        # later heads' chunk streams
                        nonlocal ph1, ph1_step
                        if ph1 is not None:
                            ph1_step += 1
                            if next(ph1, None) is None:
                                ph1 = None

                    off = min(3, nchunks - 1)
                    for i in range(nchunks):
                        emit_score(i)
                        if i == 0:
                            # the previous head's denominator matmul +
                            # reciprocal run here, hidden behind this
                            # head's first scores (engine queues are
                            # in-order: emitted earlier would stall the
                            # PE on the DVE chain)
                            run_pending(0)
                        elif i == 2:
                            # ... and the recip-broadcast two steps
                            # later, once the reciprocal has drained
                            run_pending(1)
                        if i >= off:
                            emit_pv(i - off)
                        if i >= 3 and (i < nchunks - 5
                                       or h != hlist[-1]):
                            # (zn of the previous head materializes at
                            # i == 2 via run_pending(1)); the last head
                            # reserves 8 tiles for the normalize weave
                            pull_ph1()
                        yield 9
                    for i in range(nchunks - off, nchunks):
                        yield 5
                        if h != hlist[-1]:
                            # reserve the last tiles for the normalize
                            # weave below
                            pull_ph1()
                        emit_pv(i)
                    def make_norm(h_, zps_, d01_, tail_):
                        rec = (None if tail_ else
                               workp.tile([1, 512], f16, tag="rec"))

                        def fin1():
                            if tail_:
                                # transposed denominator: 4 tiny matmuls
                                # d01^T @ ones give den per PARTITION, so
                                # the tail normalizes with per-partition
                                # scalars -- no broadcast matmul, no bsb,
                                # and a [128,4] reciprocal (130ns).
                                dpsT = ps_s()
                                for k in range(4):
                                    nc.tensor.matmul(
                                        dpsT[:, k:k + 1],
                                        lhsT=d01_[:, 128 * k:
                                                  128 * k + 128],
                                        rhs=onesm,
                                        start=(k == 0), stop=(k == 3),
                                        skip_group_check=(k > 0))
                                nc.vector.reciprocal_approx_fast(
                                    recT_sb[:], dpsT[:, 0:4])
                                return
                            dps = ps_s()
                            nc.tensor.matmul(dps[0:1, :], lhsT=onesm,
                                             rhs=d01_[:],
                                             start=True, stop=True)
                            rec32 = workp.tile([1, 512], f32,
                                               tag="rec32", bufs=1)
                            nc.vector.reciprocal_approx_fast(
                                rec32[:], dps[0:1, :])
                            nc.vector.tensor_copy(rec[:], rec32[:])

                        def fin2():
                            if tail_:
                                # unnormalized cast, sliced per st in the
                                # tail's consumption order so the first
                                # out matmul starts after ~100ns
                                z = postp.tile([128, 512], f16,
                                               tag=f"zn{h_}_{j}",
                                               name=f"zn{h_}_{j}")
                                for stc in (0, 2, 1, 3):
                                    nc.vector.tensor_copy(
                                        z[:, 128 * stc:128 * stc + 128],
                                        zps_[:, 128 * stc:
                                             128 * stc + 128])
                                zn[h_][j] = z
                                return
                            bps = ps_s()
                            nc.tensor.matmul(bps[:], lhsT=onescol,
                                             rhs=rec[:],
                                             start=True, stop=True)
                            bsb = workp.tile([128, 512], f16, tag="bsb")
                            nc.vector.tensor_copy(bsb[:], bps[:])
                            z = postp.tile([128, 512], f16,
                                           tag=f"zn{h_}_{j}",
                                           name=f"zn{h_}_{j}")
                            nc.vector.tensor_mul(z[:], zps_[:], bsb[:])
                            zn[h_][j] = z
                        return fin1, fin2

                    pending_norm[0], pending_norm[1] = make_norm(
                        h, zps, den_state["dsum01"],
                        two_phase_out and h == hlist[-1])
                    yield 9
                    if two_phase_out and h == 2:
                        ph1 = _out_phase12_gen(j)
                        ph1_step = 0
                # last head: weave leftover out-tiles between the
                # denominator matmul and the broadcast so the PE is not
                # idle while the reciprocal drains on the DVE
                run_pending(0)
                for _ in range(4):
                    if ph1 is not None and next(ph1, None) is None:
                        ph1 = None
                run_pending(1)
                while ph1 is not None:
                    if next(ph1, None) is None:
                        ph1 = None

            osb12 = osbp.tile([128, 4, 4, 512], f16, tag="osb12",
                              name="osb12", bufs=1)

            def _out_row_dma(j, st, accum, half=None):
                # overwrite + accumulate row-DMAs share the gpsimd SWDGE
                # queue: single issue path => per-row ordering guaranteed
                # (only SWDGE can do accumulating DMAs).  half=0/1 sends
                # a [128,1024] half-row so the slow read-modify-write
                # transfer overlaps the remaining casts.
                row = j * 512 + st * 128
                op = (mybir.AluOpType.add if accum
                      else mybir.AluOpType.bypass)
                if half is None:
                    nc.gpsimd.dma_start(
                        outp_d[row:row + 128, :], osb12[:, st, :, :],
                        accum_op=op)
                else:
                    c0 = half * 1024
                    nc.gpsimd.dma_start(
                        outp_d[row:row + 128, c0:c0 + 1024],
                        osb12[:, st, 2 * half:2 * half + 2, :],
                        accum_op=op)

            def _out_phase12_gen(j):
                # heads 0-2 of the final out-block: pure-PE filler pulled
                # into the last attention head's chunk stream
                for st in range(4):
                    for db in range(4):
                        ops = ps_gen()
                        for h in range(3):
                            nc.tensor.matmul(
                                ops[:],
                                lhsT=zn[h][j][:, st * 128:(st + 1) * 128],
                                rhs=woT_sb[:, h, db * 512:(db + 1) * 512],
                                start=(h == 0), stop=(h == 2))
                        nc.vector.tensor_copy(osb12[:, st, db, :], ops[:])
                        if db == 3 and st >= 2:
                            # rows 2-3 take the DMA-accumulate path; rows
                            # 0-1 combine in SBUF at the tail instead
                            _out_row_dma(j, st, accum=False)
                        yield 7

            def out_block_gen(j, two_phase=False):
                for st in ([0, 2, 1, 3] if two_phase else range(4)):
                    for db in range(4):
                        # the tail runs after all attention: the 4-deep
                        # s pool is idle, use it for a deeper pipeline
                        ops = ps_s() if two_phase else ps_gen()
                        h0 = 3 if two_phase else 0
                        for h in range(h0, HPC):
                            nc.tensor.matmul(
                                ops[:],
                                lhsT=zn[h][j][:, st * 128:(st + 1) * 128],
                                rhs=woT_sb[:, h, db * 512:(db + 1) * 512],
                                start=(h == h0), stop=(h == HPC - 1))
                        if two_phase:
                            # split tail: accumulate-DMA moves data at
                            # ~1/3 rate (read-modify-write), so only rows
                            # 2-3 take it (ScalarE casts, it is exp-free
                            # now); rows 0-1 combine on the DVE and go
                            # out as plain DMAs -- two independent
                            # streams in parallel.
                            if st >= 2:
                                nc.scalar.mul(osb12[:, st, db, :],
                                              ops[:],
                                              recT_sb[:, st:st + 1])
                                if db == 3:
                                    _out_row_dma(j, st, accum=True)
                            else:
                                osb = osbp.tile([128, 512], f16,
                                                tag="osb", name="osb",
                                                bufs=6)
                                nc.vector.scalar_tensor_tensor(
                                    osb[:], ops[:],
                                    recT_sb[:, st:st + 1],
                                    osb12[:, st, db, :],
                                    mybir.AluOpType.mult,
                                    mybir.AluOpType.add)
                                row = j * 512 + st * 128
                                nc.sync.dma_start(
                                    outp_d[row:row + 128,
                                           db * 512:(db + 1) * 512],
                                    osb[:])
                            yield 5
                        else:
                            osb = osbp.tile([128, 512], f16, tag="osb",
                                            name="osb", bufs=6)
                            if (st * 4 + db) % 2 == 0:
                                nc.scalar.copy(osb[:], ops[:])
                            else:
                                nc.vector.tensor_copy(osb[:], ops[:])
                            row = j * 512 + st * 128
                            nc.sync.dma_start(
                                outp_d[row:row + 128,
                                       db * 512:(db + 1) * 512], osb[:])
                            yield 18

            def chain_gens(*gens):
                for g in gens:
                    for w in g:
                        yield w

            def drive(main_gen, main_total, side_gen, side_total):
                """Interleave side yields into main, paced by the
                PE-time weights both generators yield."""
                mw = 0
                sw = 0
                done = side_total == 0
                for w in main_gen:
                    mw += w
                    while not done and sw * main_total < mw * side_total:
                        try:
                            sw += next(side_gen)
                        except StopIteration:
                            done = True
                while not done:
                    try:
                        next(side_gen)
                    except StopIteration:
                        done = True

            # ---- Projections: xT + W streamed through a scoped pool
            with tc.tile_pool(name="big", bufs=1) as bigp:
                w_sb = {}
                for name in ("wq", "wk", "wv"):
                    w_sb[name] = bigp.tile([128, NC_CHUNKS, HE], f16,
                                           tag=name, name=name + "_sb")
                wsrc = {name: dram.rearrange("(c p) n -> p c n", p=128)
                        for name, dram in
                        (("wq", wq_d), ("wk", wk_d), ("wv", wv_d))}
                xsrc = xT_d.rearrange("(c p) s -> p c s", p=128)

                def xT_tile():
                    return bigp.tile([128, NC_CHUNKS, 512], f16,
                                     tag="xT", name="xT", bufs=2)

                # DMA emission in wave-A consumption order, with the
                # issue cost spread across three engine queues (sync,
                # scalar, vector) so the ~650ns-per-issue sequencer cost
                # doesn't pace the stream.  The back half of each tensor
                # goes out as one batched DMA (descriptors spread over
                # all 16 DMA engines either way).
                # The per-chunk j0 stream goes out FIRST, split across the
                # sync and scalar HWDGE queues (~650ns per issue each, so
                # the two queues together outpace the PE's ~1.5us/chunk
                # consumption).  Bulk transfers are emitted strictly
                # AFTER it: DMA engines drain descriptors in arrival
                # order, so early bulk would starve the chunk stream
                # (measured: 5.4us PE gap + HAM re-throttle).
                xT0 = xT_tile()
                for c in range(8):
                    qa = nc.sync if c % 2 == 0 else nc.scalar
                    qb = nc.scalar if c % 2 == 0 else nc.sync
                    qa.dma_start(w_sb["wk"][:, c:c + 1, :],
                                 wsrc["wk"][:, c:c + 1, :])
                    qb.dma_start(w_sb["wq"][:, c:c + 1, :],
                                 wsrc["wq"][:, c:c + 1, :])
                    qa.dma_start(xT0[:, c:c + 1, :],
                                 xsrc[:, c:c + 1, 0:512])
                # Everything else strictly AFTER the chunk stream, still
                # on the two HWDGE queues, in first-use order, at 2-4
                # chunk granularity: DIRECT2D blocks its sequencer for
                # ~0.65us per 128 descriptors, so one huge DMA would
                # stall the queue; and the 16 DMA engines drain their
                # descriptor FIFOs in arrival order, so emission order IS
                # arrival order.  No gpsimd SWDGE for inputs: its desc
                # generation runs as soon as the idle gpsimd sequencer
                # reaches it, jumping ahead of the chunk stream.
                alt = [nc.sync, nc.scalar]

                def adma(i, dst, src):
                    alt[i % 2].dma_start(dst, src)

                for g in range(4):
                    lo, hi = 8 + 2 * g, 10 + 2 * g
                    adma(g, xT0[:, lo:hi, :], xsrc[:, lo:hi, 0:512])
                    adma(g + 1, w_sb["wk"][:, lo:hi, :],
                         wsrc["wk"][:, lo:hi, :])
                    adma(g, w_sb["wq"][:, lo:hi, :],
                         wsrc["wq"][:, lo:hi, :])
                for g in range(4):
                    lo, hi = 4 * g, 4 * g + 4
                    adma(g, w_sb["wv"][:, lo:hi, :],
                         wsrc["wv"][:, lo:hi, :])
                nc.sync.dma_start(masks_sb[:], masks_d[:])
                xTb = [xT0, None, None, None]
                wot_src = woT_d.rearrange("(c p) d -> p c d", p=128)
                for j in range(1, NBLK):
                    xTb[j] = xT_tile()
                    for q in range(4):
                        adma(j + q, xTb[j][:, 4 * q:4 * q + 4, :],
                             xsrc[:, 4 * q:4 * q + 4,
                                  j * 512:(j + 1) * 512])
                nc.sync.dma_start(woT_sb[:, 0:2, :], wot_src[:, 0:2, :])
                nc.scalar.dma_start(woT_sb[:, 2:4, :], wot_src[:, 2:4, :])

                # ---- j0: column-interleaved waves so the PE consumes
                # chunks at DMA arrival rate from the first piece.
                def wave(chains, blips=0, drain_order=None):
                    # chains: list of (psum_tile, lhsT_of_c, dst_ap).
                    # blips: extra tiny matmuls after the first steps —
                    # they fill the DMA-wait gaps so the HAM activity
                    # window stays busy and the clock gate opens early.
                    for c in range(NC_CHUNKS - 1):
                        for ps, lhs_of_c, _ in chains:
                            nc.tensor.matmul(
                                ps[:], lhsT=lhs_of_c(c),
                                rhs=xT0[:, c, :],
                                start=(c == 0), stop=False)
                        if c < blips:
                            for _ in range(2):
                                wps = ps_s()
                                nc.tensor.matmul(
                                    wps[0:1, 0:128], lhsT=onesm,
                                    rhs=ones_sb[:, 1:129],
                                    start=True, stop=True)
                    # last chunk + cast per chain, so the drains overlap
                    # the next wave's first matmuls instead of stalling
                    # it; drain in the NEXT wave's bank-need order (wave-B
                    # grabs z first, then gen, then s)
                    for k, (ps, lhs_of_c, dst) in enumerate(
                            drain_order or chains):
                        nc.tensor.matmul(
                            ps[:], lhsT=lhs_of_c(NC_CHUNKS - 1),
                            rhs=xT0[:, NC_CHUNKS - 1, :],
                            start=False, stop=True)
                        if k % 2 == 0:
                            nc.scalar.copy(dst, ps[:])
                        else:
                            nc.vector.tensor_copy(dst, ps[:])

                def wslice(name, h):
                    return lambda c: w_sb[name][:, c, h * E:(h + 1) * E]

                waveA = []
                for h in range(HPC):
                    waveA.append((ps_gen() if h < 2 else ps_s(),
                                  wslice("wk", h), kT[h][:, 0:512]))
                for h in range(3):
                    waveA.append((ps_s() if h == 0 else ps_z(),
                                  wslice("wq", h), qT[h][:, 0:512]))
                wave(waveA, blips=4,
                     drain_order=[waveA[5], waveA[6], waveA[0],
                                  waveA[1], waveA[2], waveA[3],
                                  waveA[4]])

                # wave-B: qT h3 + the four v tiles of block 0.
                # v chains: lhsT = xT chunk slice, rhs = wv chunk.
                vB = []
                for m in range(4):
                    vB.append((ps_gen() if m < 2 else ps_s(), m))
                psq3 = ps_z()
                for c in range(NC_CHUNKS - 1):
                    nc.tensor.matmul(
                        psq3[:], lhsT=wslice("wq", 3)(c),
                        rhs=xT0[:, c, :],
                        start=(c == 0), stop=False)
                    for ps, m in vB:
                        nc.tensor.matmul(
                            ps[:], lhsT=xT0[:, c, m * 128:(m + 1) * 128],
                            rhs=w_sb["wv"][:, c, :],
                            start=(c == 0), stop=False)
                cl = NC_CHUNKS - 1
                nc.tensor.matmul(psq3[:], lhsT=wslice("wq", 3)(cl),
                                 rhs=xT0[:, cl, :], start=False, stop=True)
                nc.scalar.copy(qT[3][:, 0:512], psq3[:])
                for ps, m in vB:
                    nc.tensor.matmul(
                        ps[:], lhsT=xT0[:, cl, m * 128:(m + 1) * 128],
                        rhs=w_sb["wv"][:, cl, :], start=False, stop=True)
                    if m % 2 == 0:
                        nc.scalar.copy(vt[m][:], ps[:])
                    else:
                        nc.vector.tensor_copy(vt[m][:], ps[:])

                # ---- j1..j3 projections, interleaved with the previous
                # block's attention + output projection.
                def proj_block_gen(j):
                    def proj(dst_ap, lhs_of_c, rhs_of_c):
                        ps = ps_gen()
                        for c in range(NC_CHUNKS):
                            nc.tensor.matmul(
                                ps[:], lhsT=lhs_of_c(c), rhs=rhs_of_c(c),
                                start=(c == 0), stop=(c == NC_CHUNKS - 1))
                        nc.vector.tensor_copy(dst_ap, ps[:])

                    for h in range(HPC):
                        for dst, w in ((kT[h], "wk"), (qT[h], "wq")):
                            proj(dst[:, j * 512:(j + 1) * 512],
                                 wslice(w, h),
                                 lambda c: xTb[j][:, c, :])
                            yield 34
                    for m in range(4 * j, 4 * j + 4):
                        proj(vt[m][:],
                             lambda c, m=m: xTb[j][:, c,
                                                   (m - 4 * j) * 128:
                                                   (m - 4 * j + 1) * 128],
                             lambda c: w_sb["wv"][:, c, :])
                        yield 34

                # stages: proj(j) interleaved with attn(j-1) — the
                # out-blocks (and attn(2)'s last head, for balance) are
                # saved for the final stretch, where they are the
                # PE-dense filler for attn(3)'s exp-paced stream.
                for j in range(1, NBLK):
                    nheads = 3 if j == 3 else 4
                    nch = 4 * (j - 1) + 4
                    side_total = nheads * (nch * 9 + 3 * 5 + 9)
                    drive(proj_block_gen(j), 34 * 12,
                          attn_block_gen(j - 1, heads=range(nheads)),
                          side_total)

            # ---- final stretch: [attn(2)h3 + attn(3)] ∥ out(0..2),
            # then out(3) ph2.  Heads stay sequential (single dps bank).
            main = chain_gens(attn_block_gen(2, heads=[3]),
                              attn_block_gen(NBLK - 1,
                                             two_phase_out=True))
            side = chain_gens(out_block_gen(0), out_block_gen(1),
                              out_block_gen(2))
            nch = 4 * (NBLK - 1) + 4
            main_total = 4 * (nch * 9 + 3 * 5 + 9) + (12 * 9 + 3 * 5 + 9)
            drive(main, main_total, side, 48 * 18)
            for _ in out_block_gen(NBLK - 1, two_phase=True):
                pass

    nc.compile()
    return nc


def _get_nc():
    if "nc" not in _CACHE:
        _CACHE["nc"] = _build_program()
    return _CACHE["nc"]


def _host_inputs(x, W_Q, W_K, W_V, W_O):
    """Per-core input dicts (all fp16, pre-transposed)."""
    cc = np.arange(128)[None, :]
    mm = np.arange(128)[:, None]
    masks = (cc >= mm).astype(np.float16)   # [128,128] diagonal band
    in_maps = []
    for c in range(N_CORES):
        b, g = divmod(c, 4)
        hs = slice(HPC * g, HPC * g + HPC)
        xT = np.ascontiguousarray(x[b].T).astype(np.float16)
        wq = np.ascontiguousarray(
            W_Q[hs].transpose(2, 0, 1).reshape(D, HE)).astype(np.float16)
        wk = np.ascontiguousarray(
            W_K[hs].transpose(2, 0, 1).reshape(D, HE)).astype(np.float16)
        wv = np.ascontiguousarray(
            W_V[hs].transpose(2, 0, 1).reshape(D, HE)).astype(np.float16)
        woT = np.ascontiguousarray(
            W_O[hs].transpose(0, 2, 1).reshape(HE, D)).astype(np.float16)
        in_maps.append({"xT": xT, "wq": wq, "wk": wk, "wv": wv,
                        "woT": woT, "masks": masks})
    return in_maps


def _run(in_maps, trace=False, **kw):
    from concourse.bass_utils import run_bass_kernel_spmd
    nc = _get_nc()
    return run_bass_kernel_spmd(nc, in_maps, list(range(N_CORES)),
                                trace=trace, **kw)


def kernel(x, W_Q, W_K, W_V, W_O):
    x, W_Q, W_K, W_V, W_O = (np.asarray(a, dtype=np.float32)
                             for a in (x, W_Q, W_K, W_V, W_O))
    res = _run(_host_inputs(x, W_Q, W_K, W_V, W_O))
    parts = [np.asarray(res.results[c]["outp"], dtype=np.float32)
             for c in range(N_CORES)]
    out = np.stack([parts[0] + parts[1] + parts[2] + parts[3],
                    parts[4] + parts[5] + parts[6] + parts[7]])
    return out

